# revision 4
# baseline (speedup 1.0000x reference)
"""Trainium2 Bass kernel for nn_MetaSelectTarget (FPN level assignment).

Strategy (v2):
  - Data-parallel over batch: B=8 images -> 8 NeuronCores.
  - Host packs cls_pred and regr_pred*0.25 into one combined [NLOC+pad, 84]
    tensor so a single indirect-DMA gather fetches a window row of both.
  - 128-partition layout: partition g (top half) and 64+g (bottom half)
    hold the same box g; window rows are split by parity across halves
    (top: h=0,2,4,..., bottom: h=1,3,5,...), so every elementwise op runs
    at full 128-lane width.
  - 12 slot gathers (one per window row slot across all 5 levels) fill a
    uniform [128, 70 cells, 84] tile.  Focal f0 is computed from the
    gathered windows (no dense pass over all 21824 locations), with the
    masked class-sum fused into scalar_tensor_tensor accum_out ops.
  - psel (per-cell selected-class prob) via onehot multiply + reduce.
  - IoU targets use the identity stride/(4*stride)=0.25 so sx-terms are
    level-independent; per-level scale/bias folded into ACT Relu ops with
    per-partition bias APs.
"""

import numpy as np

import concourse.bass as bass
import concourse.bacc as bacc
import concourse.tile as tile
from concourse import mybir
from contextlib import ExitStack

f32 = mybir.dt.float32
i32 = mybir.dt.int32
AF = mybir.ActivationFunctionType
OP = mybir.AluOpType
AX = mybir.AxisListType

G = 64
C = 80
NLEV = 5
FS = [(128, 128), (64, 64), (32, 32), (16, 16), (8, 8)]
STRIDES = [8.0, 16.0, 32.0, 64.0, 128.0]
ROWOFS = [0, 16384, 20480, 21504, 21760]
NLOC = 21824
NPAD = NLOC + 16
W = [9, 5, 3, 2, 2]            # window size per level
NSLOT = [5, 3, 2, 1, 1]        # row slots per half per level
SLOT_R = [(0, 5), (5, 8), (8, 10), (10, 11), (11, 12)]
CELL_R = [(0, 45), (45, 60), (60, 66), (66, 68), (68, 70)]
NCELL = 70
RC = 84                        # combined row width (80 cls + 4 regr)
FREE = NCELL * RC              # 5880
ALPHA = 0.25
EPS = 1e-7
BIG = 1e7
RSCALE = 0.25                  # host-side regr scale (exact power of 2)


def build_nc(num_devices=8):
    nc = bacc.Bacc("TRN2", target_bir_lowering=False, num_devices=num_devices)

    cr_b = nc.dram_tensor("cr_b", [NPAD, RC], f32, kind="ExternalInput")
    gt_b = nc.dram_tensor("gt_b", [G, 5], f32, kind="ExternalInput")
    out_lvl = nc.dram_tensor("out_lvl", [G], i32, kind="ExternalOutput")

    # ---- inline constants -------------------------------------------------
    # per-(l, j) tables, j: 0=x1, 1=y1, 2=x2, 3=y2 (replicated both halves)
    recip = np.zeros((128, NLEV, 4), np.float32)
    maskF = np.zeros((128, NLEV, 4), np.float32)
    maskC = np.zeros((128, NLEV, 4), np.float32)
    clo = np.zeros((128, NLEV, 4), np.float32)
    chi = np.zeros((128, NLEV, 4), np.float32)
    shi = np.zeros((128, NLEV, 4), np.float32)
    for l in range(NLEV):
        fh, fw = FS[l]
        w = W[l]
        recip[:, l, :] = 1.0 / STRIDES[l]
        maskF[:, l, 0] = maskF[:, l, 1] = 1.0
        maskC[:, l, 2] = maskC[:, l, 3] = 1.0
        clo[:, l, :] = [0.0, 0.0, 1.0, 1.0]
        chi[:, l, :] = [fw - 1, fh - 1, fw, fh]
        shi[:, l, :] = [fw - w, fh - w, 1e9, 1e9]
    inv4s = np.zeros((128, NLEV), np.float32)
    ninv4s = np.zeros((128, NLEV), np.float32)
    rowofs_c = np.zeros((128, NLEV), np.float32)
    for l in range(NLEV):
        inv4s[:, l] = 1.0 / (4.0 * STRIDES[l])
        ninv4s[:, l] = -1.0 / (4.0 * STRIDES[l])
        rowofs_c[:, l] = float(ROWOFS[l])

    # per-slot true h and effective (clamped) h, per half
    h_slot = np.zeros((128, 12), np.float32)
    heff_slot = np.zeros((128, 12), np.float32)
    for l in range(NLEV):
        s0, s1 = SLOT_R[l]
        for k in range(NSLOT[l]):
            ht = 2 * k
            hb = 2 * k + 1
            h_slot[0:64, s0 + k] = ht
            h_slot[64:128, s0 + k] = hb
            heff_slot[0:64, s0 + k] = min(ht, W[l] - 1)
            heff_slot[64:128, s0 + k] = min(hb, W[l] - 1)

    # per-cell i-within-row and true h
    iota_cell = np.zeros((128, NCELL), np.float32)
    h_cell = np.zeros((128, NCELL), np.float32)
    for l in range(NLEV):
        c0, c1 = CELL_R[l]
        s0, s1 = SLOT_R[l]
        w = W[l]
        for k in range(NSLOT[l]):
            for i in range(w):
                cell = c0 + k * w + i
                iota_cell[:, cell] = i
                h_cell[:, cell] = h_slot[:, s0 + k]

    cconst = np.tile(np.arange(C, dtype=np.float32), (128, 1))
    constl = np.tile(np.arange(NLEV, dtype=np.float32), (128, 1))
    constm1 = np.full((128, 1), -1.0, np.float32)

    consts = np.concatenate(
        [recip.reshape(128, -1), maskF.reshape(128, -1), maskC.reshape(128, -1),
         clo.reshape(128, -1), chi.reshape(128, -1), shi.reshape(128, -1),
         inv4s, ninv4s, rowofs_c, h_slot, heff_slot, iota_cell, h_cell,
         cconst, constl, constm1], axis=1)
    t_consts = nc.inline_tensor(consts, "c_all")
    NCONST = consts.shape[1]

    with tile.TileContext(nc) as tc, ExitStack() as ctx:
        pc = ctx.enter_context(tc.tile_pool(name="pc", bufs=1))

        CST = pc.tile([128, NCONST], f32)
        nc.sync.dma_start(CST[:], t_consts[:])
        off = 0
        def _cv(n):
            nonlocal off
            v = CST[:, off:off + n]
            off += n
            return v
        RECIP = _cv(20); MASKF = _cv(20); MASKC = _cv(20)
        CLO = _cv(20); CHI = _cv(20); SHI = _cv(20)
        INV4S = _cv(5); NINV4S = _cv(5); ROWOF = _cv(5)
        HSLOT = _cv(12); HEFF = _cv(12)
        IOTA = _cv(NCELL); HCELL = _cv(NCELL)
        CCONST = _cv(C); CONSTL = _cv(5); CONSTM1 = _cv(1)

        GT = pc.tile([128, 5], f32)
        nc.sync.dma_start(GT[0:64, :], gt_b[:])
        nc.sync.dma_start(GT[64:128, :], gt_b[:])

        # ---- box math (both halves identical) -----------------------------
        Q = pc.tile([128, 4], f32)
        TMPQ = pc.tile([128, 4], f32)
        nc.vector.tensor_scalar(TMPQ[:, 0:1], GT[:, 2:3], 0.4, None, OP.mult)
        nc.vector.tensor_scalar(TMPQ[:, 1:2], GT[:, 3:4], 0.4, None, OP.mult)
        nc.vector.tensor_scalar(TMPQ[:, 2:3], GT[:, 0:1], 0.4, None, OP.mult)
        nc.vector.tensor_scalar(TMPQ[:, 3:4], GT[:, 1:2], 0.4, None, OP.mult)
        nc.vector.scalar_tensor_tensor(Q[:, 0:1], GT[:, 0:1], 0.6, TMPQ[:, 0:1], OP.mult, OP.add)
        nc.vector.scalar_tensor_tensor(Q[:, 1:2], GT[:, 1:2], 0.6, TMPQ[:, 1:2], OP.mult, OP.add)
        nc.vector.scalar_tensor_tensor(Q[:, 2:3], GT[:, 2:3], 0.6, TMPQ[:, 2:3], OP.mult, OP.add)
        nc.vector.scalar_tensor_tensor(Q[:, 3:4], GT[:, 3:4], 0.6, TMPQ[:, 3:4], OP.mult, OP.add)

        V = pc.tile([128, 20], f32)
        nc.vector.tensor_tensor(
            out=V[:].rearrange("g (l j) -> g l j", j=4),
            in0=Q[:, None, :].to_broadcast([128, NLEV, 4]),
            in1=RECIP.rearrange("g (l j) -> g l j", j=4),
            op=OP.mult)
        VI = pc.tile([128, 20], i32)
        nc.vector.tensor_copy(VI[:], V[:])
        VF = pc.tile([128, 20], f32)
        nc.vector.tensor_copy(VF[:], VI[:])
        GG = pc.tile([128, 20], f32)
        nc.vector.tensor_tensor(out=GG[:], in0=VF[:], in1=V[:], op=OP.is_gt)
        LL = pc.tile([128, 20], f32)
        nc.vector.tensor_tensor(out=LL[:], in0=VF[:], in1=V[:], op=OP.is_lt)
        nc.vector.tensor_tensor(out=GG[:], in0=GG[:], in1=MASKF, op=OP.mult)
        nc.vector.tensor_tensor(out=LL[:], in0=LL[:], in1=MASKC, op=OP.mult)
        VR = pc.tile([128, 20], f32)
        nc.vector.tensor_tensor(out=VR[:], in0=VF[:], in1=GG[:], op=OP.subtract)
        nc.vector.tensor_tensor(out=VR[:], in0=VR[:], in1=LL[:], op=OP.add)
        nc.vector.tensor_tensor(out=VR[:], in0=VR[:], in1=CLO, op=OP.max)
        nc.vector.tensor_tensor(out=VR[:], in0=VR[:], in1=CHI, op=OP.min)
        SV = pc.tile([128, 20], f32)
        nc.vector.tensor_tensor(out=SV[:], in0=VR[:], in1=SHI, op=OP.min)

        VR3 = VR[:].rearrange("g (l j) -> g l j", j=4)
        SV3 = SV[:].rearrange("g (l j) -> g l j", j=4)
        x1v, y1v, x2v, y2v = VR3[:, :, 0], VR3[:, :, 1], VR3[:, :, 2], VR3[:, :, 3]
        xsv, ysv = SV3[:, :, 0], SV3[:, :, 1]

        EX = pc.tile([128, 5], f32)
        nc.vector.tensor_tensor(out=EX[:], in0=x1v, in1=x2v, op=OP.is_equal)
        EY = pc.tile([128, 5], f32)
        nc.vector.tensor_tensor(out=EY[:], in0=y1v, in1=y2v, op=OP.is_equal)
        EMX = pc.tile([128, 5], f32)
        nc.vector.tensor_tensor(out=EMX[:], in0=EX[:], in1=EY[:], op=OP.max)
        DX = pc.tile([128, 5], f32)
        nc.vector.tensor_tensor(out=DX[:], in0=x2v, in1=x1v, op=OP.subtract)
        DY = pc.tile([128, 5], f32)
        nc.vector.tensor_tensor(out=DY[:], in0=y2v, in1=y1v, op=OP.subtract)
        DN = pc.tile([128, 5], f32)
        nc.vector.tensor_tensor(out=DN[:], in0=DX[:], in1=DY[:], op=OP.mult)
        nc.vector.tensor_scalar(DN[:], DN[:], 1.0, None, OP.max)
        RECDN = pc.tile([128, 5], f32)
        nc.vector.reciprocal(RECDN[:], DN[:])

        LBL = pc.tile([128, 1], f32)
        nc.vector.tensor_scalar(LBL[:], GT[:, 4:5], 0.0, float(C - 1), OP.max, OP.min)
        ONEHOT = pc.tile([128, C], f32)
        nc.vector.tensor_tensor(out=ONEHOT[:], in0=CCONST,
                                in1=LBL[:, 0:1].to_broadcast([128, C]), op=OP.is_equal)

        SABS = pc.tile([128, 1], f32)
        nc.vector.tensor_reduce(SABS[:], GT[:, 0:4], axis=AX.X, op=OP.add,
                                apply_absolute_value=True)
        NV = pc.tile([128, 1], i32)
        nc.vector.tensor_scalar(NV[:], SABS[:], 0.0, None, OP.is_le)

        # ---- row indices per slot ------------------------------------------
        XSO = pc.tile([128, 5], f32)
        nc.vector.tensor_tensor(out=XSO[:], in0=xsv, in1=ROWOF, op=OP.add)
        RIF = pc.tile([128, 12], f32)
        for l in range(NLEV):
            s0, s1 = SLOT_R[l]
            ns = s1 - s0
            nc.vector.tensor_tensor(
                out=RIF[:, s0:s1], in0=HEFF[:, s0:s1],
                in1=ysv[:, l:l + 1].to_broadcast([128, ns]), op=OP.add)
            nc.vector.scalar_tensor_tensor(
                RIF[:, s0:s1], RIF[:, s0:s1], float(FS[l][1]),
                XSO[:, l:l + 1].to_broadcast([128, ns]), OP.mult, OP.add)
        RI = pc.tile([128, 12], i32)
        nc.vector.tensor_copy(RI[:], RIF[:])

        # ---- gathers -------------------------------------------------------
        CR = pc.tile([128, FREE], f32)
        slot_off = []
        o = 0
        for l in range(NLEV):
            for k in range(NSLOT[l]):
                slot_off.append((l, o))
                o += W[l] * RC
        for s, (l, off) in enumerate(slot_off):
            nc.gpsimd.indirect_dma_start(
                out=CR[:, off:off + W[l] * RC], out_offset=None, in_=cr_b[:],
                in_offset=bass.IndirectOffsetOnAxis(ap=RI[:, s:s + 1], axis=0))

        CR3 = CR[:].rearrange("g (k c) -> g k c", c=RC)

        # ---- big class passes (per level for pipelining) -------------------
        LN1 = pc.tile([128, NCELL * C], f32)
        LN13 = LN1[:].rearrange("g (k c) -> g k c", c=C)
        SQ = pc.tile([128, NCELL * C], f32)
        SQ3 = SQ[:].rearrange("g (k c) -> g k c", c=C)
        OHM = pc.tile([128, NCELL * C], f32)
        OHM3 = OHM[:].rearrange("g (k c) -> g k c", c=C)
        PSEL = pc.tile([128, NCELL], f32)
        S1 = pc.tile([128, 5], f32)
        MXY = pc.tile([128, NCELL], f32)
        TMC = pc.tile([128, NCELL], f32)

        # masks first (cheap; needed by S1 accumulation)
        XSI = pc.tile([128, NCELL], f32)
        HYC = pc.tile([128, NCELL], f32)
        for l in range(NLEV):
            c0, c1 = CELL_R[l]
            ncl = c1 - c0
            nc.vector.tensor_tensor(
                out=XSI[:, c0:c1], in0=IOTA[:, c0:c1],
                in1=xsv[:, l:l + 1].to_broadcast([128, ncl]), op=OP.add)
            nc.vector.tensor_tensor(
                out=HYC[:, c0:c1], in0=HCELL[:, c0:c1],
                in1=ysv[:, l:l + 1].to_broadcast([128, ncl]), op=OP.add)
            nc.vector.tensor_scalar(MXY[:, c0:c1], XSI[:, c0:c1],
                                    VR[:, 4 * l:4 * l + 1], None, OP.is_ge)
            nc.vector.tensor_scalar(TMC[:, c0:c1], XSI[:, c0:c1],
                                    VR[:, 4 * l + 2:4 * l + 3], None, OP.is_lt)
        nc.vector.tensor_tensor(out=MXY[:], in0=MXY[:], in1=TMC[:], op=OP.mult)
        TMC2 = pc.tile([128, NCELL], f32)
        for l in range(NLEV):
            c0, c1 = CELL_R[l]
            nc.vector.tensor_scalar(TMC[:, c0:c1], HYC[:, c0:c1],
                                    VR[:, 4 * l + 1:4 * l + 2], None, OP.is_ge)
            nc.vector.tensor_scalar(TMC2[:, c0:c1], HYC[:, c0:c1],
                                    VR[:, 4 * l + 3:4 * l + 4], None, OP.is_lt)
        nc.vector.tensor_tensor(out=TMC[:], in0=TMC[:], in1=TMC2[:], op=OP.mult)
        nc.vector.tensor_tensor(out=MXY[:], in0=MXY[:], in1=TMC[:], op=OP.mult)

        for l in range(NLEV):
            c0, c1 = CELL_R[l]
            ncl = c1 - c0
            cls3 = CR3[:, c0:c1, 0:C]
            nc.scalar.activation(LN13[:, c0:c1, :], cls3, AF.Ln, bias=1.0, scale=-1.0)
            nc.scalar.activation(SQ3[:, c0:c1, :], cls3, AF.Square)
            # T = sq * ln1  (in place into SQ)
            nc.vector.tensor_tensor(out=SQ[:, c0 * C:c1 * C], in0=SQ[:, c0 * C:c1 * C],
                                    in1=LN1[:, c0 * C:c1 * C], op=OP.mult)
            # S1_l = sum over (cells, classes) of mask * T
            nc.vector.scalar_tensor_tensor(
                SQ3[:, c0:c1, :], SQ3[:, c0:c1, :], 1.0,
                MXY[:, c0:c1, None].to_broadcast([128, ncl, C]),
                OP.mult, OP.mult, accum_out=S1[:, l:l + 1])
            # onehot select
            nc.gpsimd.tensor_tensor(
                out=OHM3[:, c0:c1, :], in0=cls3,
                in1=ONEHOT[:, None, :].to_broadcast([128, ncl, C]), op=OP.mult)
            nc.vector.tensor_reduce(PSEL[:, c0:c1], OHM3[:, c0:c1, :],
                                    axis=AX.X, op=OP.add)

        # ---- psel focal terms ---------------------------------------------
        LNP = pc.tile([128, NCELL], f32)
        nc.scalar.activation(LNP[:], PSEL[:], AF.Ln)
        LN1P = pc.tile([128, NCELL], f32)
        nc.scalar.activation(LN1P[:], PSEL[:], AF.Ln, bias=1.0, scale=-1.0)
        SQ1P = pc.tile([128, NCELL], f32)
        nc.scalar.activation(SQ1P[:], PSEL[:], AF.Square, bias=1.0, scale=-1.0)
        SQP = pc.tile([128, NCELL], f32)
        nc.scalar.activation(SQP[:], PSEL[:], AF.Square)
        M1 = pc.tile([128, NCELL], f32)
        nc.vector.tensor_tensor(out=M1[:], in0=SQ1P[:], in1=LNP[:], op=OP.mult)
        M2 = pc.tile([128, NCELL], f32)
        nc.vector.tensor_tensor(out=M2[:], in0=SQP[:], in1=LN1P[:], op=OP.mult)

        # ---- iou terms ------------------------------------------------------
        # sx*inv4s = (xsi + 0.5) * 0.25 (level-independent)
        SXS = pc.tile([128, NCELL], f32)
        nc.vector.tensor_scalar(SXS[:], XSI[:], 0.5, 0.25, OP.add, OP.mult)
        SYS = pc.tile([128, NCELL], f32)
        nc.vector.tensor_scalar(SYS[:], HYC[:], 0.5, 0.25, OP.add, OP.mult)
        # per-level bias tiles
        B0N = pc.tile([128, 5], f32)
        nc.vector.tensor_tensor(out=B0N[:], in0=GT[:, 0:1].to_broadcast([128, 5]),
                                in1=NINV4S, op=OP.mult)
        B1N = pc.tile([128, 5], f32)
        nc.vector.tensor_tensor(out=B1N[:], in0=GT[:, 1:2].to_broadcast([128, 5]),
                                in1=NINV4S, op=OP.mult)
        B2P = pc.tile([128, 5], f32)
        nc.vector.tensor_tensor(out=B2P[:], in0=GT[:, 2:3].to_broadcast([128, 5]),
                                in1=INV4S, op=OP.mult)
        B3P = pc.tile([128, 5], f32)
        nc.vector.tensor_tensor(out=B3P[:], in0=GT[:, 3:4].to_broadcast([128, 5]),
                                in1=INV4S, op=OP.mult)
        TL = pc.tile([128, NCELL], f32)
        TR = pc.tile([128, NCELL], f32)
        TT = pc.tile([128, NCELL], f32)
        TB = pc.tile([128, NCELL], f32)
        for l in range(NLEV):
            c0, c1 = CELL_R[l]
            nc.scalar.activation(TL[:, c0:c1], SXS[:, c0:c1], AF.Relu,
                                 bias=B0N[:, l:l + 1], scale=1.0)
            nc.scalar.activation(TR[:, c0:c1], SXS[:, c0:c1], AF.Relu,
                                 bias=B2P[:, l:l + 1], scale=-1.0)
            nc.scalar.activation(TT[:, c0:c1], SYS[:, c0:c1], AF.Relu,
                                 bias=B1N[:, l:l + 1], scale=1.0)
            nc.scalar.activation(TB[:, c0:c1], SYS[:, c0:c1], AF.Relu,
                                 bias=B3P[:, l:l + 1], scale=-1.0)

        TLR = pc.tile([128, NCELL], f32)
        nc.vector.tensor_tensor(out=TLR[:], in0=TL[:], in1=TR[:], op=OP.add)
        TTB = pc.tile([128, NCELL], f32)
        nc.vector.tensor_tensor(out=TTB[:], in0=TT[:], in1=TB[:], op=OP.add)
        TAREA = pc.tile([128, NCELL], f32)
        nc.vector.tensor_tensor(out=TAREA[:], in0=TLR[:], in1=TTB[:], op=OP.mult)

        spl = CR3[:, :, 80]
        spt = CR3[:, :, 81]
        spr = CR3[:, :, 82]
        spb = CR3[:, :, 83]
        W1 = pc.tile([128, NCELL], f32)
        nc.vector.scalar_tensor_tensor(W1[:], spl, 4.0, TL[:], OP.mult, OP.min)
        W2 = pc.tile([128, NCELL], f32)
        nc.vector.scalar_tensor_tensor(W2[:], spr, 4.0, TR[:], OP.mult, OP.min)
        WI = pc.tile([128, NCELL], f32)
        nc.vector.tensor_tensor(out=WI[:], in0=W1[:], in1=W2[:], op=OP.add)
        H1 = pc.tile([128, NCELL], f32)
        nc.vector.scalar_tensor_tensor(H1[:], spt, 4.0, TT[:], OP.mult, OP.min)
        H2 = pc.tile([128, NCELL], f32)
        nc.vector.scalar_tensor_tensor(H2[:], spb, 4.0, TB[:], OP.mult, OP.min)
        HI = pc.tile([128, NCELL], f32)
        nc.gpsimd.tensor_tensor(out=HI[:], in0=H1[:], in1=H2[:], op=OP.add)

        PLR = pc.tile([128, NCELL], f32)
        nc.gpsimd.tensor_tensor(out=PLR[:], in0=spl, in1=spr, op=OP.add)
        PTB = pc.tile([128, NCELL], f32)
        nc.gpsimd.tensor_tensor(out=PTB[:], in0=spt, in1=spb, op=OP.add)
        PAREA = pc.tile([128, NCELL], f32)
        nc.vector.scalar_tensor_tensor(PAREA[:], PLR[:], 16.0, PTB[:], OP.mult, OP.mult)

        AI = pc.tile([128, NCELL], f32)
        nc.vector.tensor_tensor(out=AI[:], in0=WI[:], in1=HI[:], op=OP.mult)
        AU = pc.tile([128, NCELL], f32)
        nc.vector.tensor_tensor(out=AU[:], in0=TAREA[:], in1=PAREA[:], op=OP.add)
        nc.vector.tensor_tensor(out=AU[:], in0=AU[:], in1=AI[:], op=OP.subtract)
        nc.vector.tensor_scalar(AI[:], AI[:], EPS, None, OP.add)
        nc.vector.tensor_scalar(AU[:], AU[:], EPS, None, OP.add)
        RAU = pc.tile([128, NCELL], f32)
        nc.vector.reciprocal(RAU[:], AU[:])
        RT = pc.tile([128, NCELL], f32)
        nc.vector.tensor_tensor(out=RT[:], in0=AI[:], in1=RAU[:], op=OP.mult)
        LNR = pc.tile([128, NCELL], f32)
        nc.scalar.activation(LNR[:], RT[:], AF.Ln)

        # SMALL = 0.25*m1 - 0.75*m2 + lnr ; S2_l = sum mask*SMALL
        SM = pc.tile([128, NCELL], f32)
        nc.vector.scalar_tensor_tensor(SM[:], M1[:], ALPHA, LNR[:], OP.mult, OP.add)
        nc.vector.scalar_tensor_tensor(SM[:], M2[:], -0.75, SM[:], OP.mult, OP.add)
        S2 = pc.tile([128, 5], f32)
        for l in range(NLEV):
            c0, c1 = CELL_R[l]
            nc.vector.scalar_tensor_tensor(
                SM[:, c0:c1], SM[:, c0:c1], 1.0, MXY[:, c0:c1],
                OP.mult, OP.mult, accum_out=S2[:, l:l + 1])

        # loss_half = (0.75*S1 + S2) * recdn
        LS = pc.tile([128, 5], f32)
        nc.vector.scalar_tensor_tensor(LS[:], S1[:], 0.75, S2[:], OP.mult, OP.add)
        nc.vector.tensor_tensor(out=LS[:], in0=LS[:], in1=RECDN[:], op=OP.mult)

        # combine halves: LOSS = EMX*BIG - (LS_top + LS_bottom)
        TMH = pc.tile([64, 5], f32)
        nc.sync.dma_start(TMH[:], LS[64:128, :])
        LC = pc.tile([64, 5], f32)
        nc.vector.tensor_tensor(out=LC[:], in0=LS[0:64, :], in1=TMH[:], op=OP.add)
        LOSS = pc.tile([64, 5], f32)
        nc.vector.scalar_tensor_tensor(LOSS[:], EMX[0:64, :], BIG, LC[:],
                                       OP.mult, OP.subtract)

        # ---- argmin over levels + valid masking ---------------------------
        MBEST = pc.tile([64, 1], f32)
        nc.vector.tensor_copy(MBEST[:], LOSS[:, 0:1])
        IDX = pc.tile([64, 1], f32)
        nc.vector.memset(IDX[:], 0.0)
        for l in range(1, NLEV):
            CMP = pc.tile([64, 1], i32, tag=f"cmp{l}")
            nc.vector.tensor_tensor(out=CMP[:], in0=LOSS[:, l:l + 1], in1=MBEST[:], op=OP.is_lt)
            nc.vector.copy_predicated(IDX[:], CMP[:], CONSTL[0:64, l:l + 1])
            nc.vector.tensor_tensor(out=MBEST[:], in0=MBEST[:], in1=LOSS[:, l:l + 1], op=OP.min)
        nc.vector.copy_predicated(IDX[:], NV[0:64, :], CONSTM1[0:64, :])
        IDXI = pc.tile([64, 1], i32)
        nc.vector.tensor_copy(IDXI[:], IDX[:])
        nc.sync.dma_start(out_lvl.ap()[:, None], IDXI[:])

    nc.compile()
    return nc


_NC_CACHE = None


def _get_nc():
    global _NC_CACHE
    if _NC_CACHE is None:
        _NC_CACHE = build_nc(num_devices=8)
    return _NC_CACHE


def make_in_maps(cls_pred, regr_pred, gt_boxes):
    B = cls_pred.shape[0]
    comb = np.zeros((B, NPAD, RC), np.float32)
    comb[:, :NLOC, 0:C] = cls_pred
    comb[:, :NLOC, C:RC] = regr_pred * RSCALE
    return [
        {"cr_b": np.ascontiguousarray(comb[b]),
         "gt_b": np.ascontiguousarray(gt_boxes[b], dtype=np.float32)}
        for b in range(B)
    ]


def kernel(cls_pred, regr_pred, feature_shapes, gt_boxes):
    from concourse.bass_utils import run_bass_kernel_spmd

    B = cls_pred.shape[0]
    assert B == 8 and cls_pred.shape[1] == NLOC and cls_pred.shape[2] == C
    nc = _get_nc()
    in_maps = make_in_maps(cls_pred, regr_pred, gt_boxes)
    res = run_bass_kernel_spmd(nc, in_maps, list(range(B)))
    out = np.stack([np.asarray(res.results[b]["out_lvl"]).reshape(G) for b in range(B)])
    return out.reshape(-1).astype(np.int32)


# revision 8
# speedup vs baseline: 1.1546x; 1.1546x over previous
"""Trainium2 Bass kernel for nn_MetaSelectTarget (FPN level assignment).

v3 design notes:
  - Data-parallel over batch: B=8 images -> 8 NeuronCores.
  - Host packs cls_pred and regr_pred*0.25 into one combined [NLOC+pad, 84]
    tensor so one indirect-DMA gather fetches a window row of both.
  - 128-partition layout: partition g (top) and 64+g (bottom) hold box g;
    window rows split across halves so ops run 128 lanes wide.
  - 10 slot gathers (L0: 5 single-row, L1: 2-row block + 1 row, L2: one
    2-row block, L3/L4: one row each), issued as early as possible on a
    minimal xs/ys critical path.
  - ACT engine is the layout normalizer: it reads the strided gathered
    tile and writes packed ln(1-p), p^2 and regr (x4 unscale folded into
    the ACT copy).  DVE only ever touches packed tiles.
  - Focal: S1_l = sum(mask * p^2 ln(1-p)) fused via scalar_tensor_tensor
    accum_out.  psel via onehot on p^2: psel^2 = sum(SQ*onehot), then
    psel = sqrt, ln(psel) = 0.5*ln(psel^2).
  - IoU: ln(ai/au) = ln(ai+eps) - ln(au+eps); elementwise chain on Pool
    (free after the gathers) + ACT.
"""

import numpy as np

import concourse.bass as bass
import concourse.bacc as bacc
import concourse.tile as tile
from concourse import mybir
from contextlib import ExitStack

f32 = mybir.dt.float32
i32 = mybir.dt.int32
AF = mybir.ActivationFunctionType
OP = mybir.AluOpType
AX = mybir.AxisListType

G = 64
C = 80
NLEV = 5
FS = [(128, 128), (64, 64), (32, 32), (16, 16), (8, 8)]
STRIDES = [8.0, 16.0, 32.0, 64.0, 128.0]
ROWOFS = [0, 16384, 20480, 21504, 21760]
NLOC = 21824
NPAD = NLOC + 16
W = [9, 5, 3, 2, 2]
CELL_R = [(0, 45), (45, 60), (60, 66), (66, 68), (68, 70)]
NCELL = 70
RC = 84
ALPHA = 0.25
EPS = 1e-7
BIG = 1e7
RSCALE = 0.25

# gathers: (level, base_h_top, base_h_bottom, block_els, CR offset)
# L0: 5 single-row gathers; L1: one 2-row block + one row; L2: one 2-row
# block; L3, L4: one row (bottom base clamped to stay in level).
GATHERS = [
    (0, 0, 1, 9 * RC, 0),
    (0, 2, 3, 9 * RC, 756),
    (0, 4, 5, 9 * RC, 1512),
    (0, 6, 7, 9 * RC, 2268),
    (0, 8, 8, 9 * RC, 3024),          # bottom dup (h=9 masked)
    (1, 0, 2, (64 + 5) * RC, 3780),   # rows h,h+1 per half
    (1, 4, 4, 5 * RC, 9576),          # bottom dup (h=5 masked)
    (2, 0, 2, (32 + 3) * RC, 9996),   # rows h,h+1 (bottom h3 masked)
    (3, 0, 1, 2 * RC, 12936),
    (4, 0, 1, 2 * RC, 13104),
]
FREE = 13272

# cell -> (true h per half, CR element offset) tables
def _cell_tables():
    h_top = np.zeros(NCELL, np.float32)
    h_bot = np.zeros(NCELL, np.float32)
    off = np.zeros(NCELL, np.int64)
    iota = np.zeros(NCELL, np.float32)
    # L0 cells 0..44
    for k in range(5):
        for i in range(9):
            c = 9 * k + i
            h_top[c] = 2 * k
            h_bot[c] = 2 * k + 1      # k=4 -> h=9, masked
            off[c] = 756 * k + i * RC
            iota[c] = i
    # L1 cells 45..59
    for i in range(5):
        c = 45 + i
        h_top[c] = 0; h_bot[c] = 2
        off[c] = 3780 + i * RC
        iota[c] = i
    for i in range(5):
        c = 50 + i
        h_top[c] = 1; h_bot[c] = 3
        off[c] = 3780 + (64 + i) * RC
        iota[c] = i
    for i in range(5):
        c = 55 + i
        h_top[c] = 4; h_bot[c] = 5    # bottom masked
        off[c] = 9576 + i * RC
        iota[c] = i
    # L2 cells 60..65
    for i in range(3):
        c = 60 + i
        h_top[c] = 0; h_bot[c] = 2
        off[c] = 9996 + i * RC
        iota[c] = i
    for i in range(3):
        c = 63 + i
        h_top[c] = 1; h_bot[c] = 3    # bottom masked
        off[c] = 9996 + (32 + i) * RC
        iota[c] = i
    # L3 cells 66..67, L4 cells 68..69
    for i in range(2):
        c = 66 + i
        h_top[c] = 0; h_bot[c] = 1
        off[c] = 12936 + i * RC
        iota[c] = i
    for i in range(2):
        c = 68 + i
        h_top[c] = 0; h_bot[c] = 1
        off[c] = 13104 + i * RC
        iota[c] = i
    return h_top, h_bot, off, iota

H_TOP, H_BOT, CELL_OFF, IOTA_NP = _cell_tables()

# contiguous runs of cells with uniform stride, for strided ACT views:
# (cell0, ncells, CR offset, row stride) where cells are i*RC apart
ACT_RUNS = [
    (0, 45, 0, RC),            # L0 (5 slots x 9 cells, uniform 756 = 9*RC)
    (45, 5, 3780, RC),
    (50, 5, 3780 + 64 * RC, RC),
    (55, 5, 9576, RC),
    (60, 3, 9996, RC),
    (63, 3, 9996 + 32 * RC, RC),
    (66, 2, 12936, RC),
    (68, 2, 13104, RC),
]


def build_nc(num_devices=8):
    nc = bacc.Bacc("TRN2", target_bir_lowering=False, num_devices=num_devices)

    cr_b = nc.dram_tensor("cr_b", [NPAD, RC], f32, kind="ExternalInput")
    gt_b = nc.dram_tensor("gt_b", [G, 5], f32, kind="ExternalInput")
    out_lvl = nc.dram_tensor("out_lvl", [G], i32, kind="ExternalOutput")

    # ---- inline constants -------------------------------------------------
    recip = np.zeros((128, NLEV, 4), np.float32)
    maskF = np.zeros((128, NLEV, 4), np.float32)
    maskC = np.zeros((128, NLEV, 4), np.float32)
    clo = np.zeros((128, NLEV, 4), np.float32)
    chi = np.zeros((128, NLEV, 4), np.float32)
    shi = np.zeros((128, NLEV, 4), np.float32)
    for l in range(NLEV):
        fh, fw = FS[l]
        w = W[l]
        recip[:, l, :] = 1.0 / STRIDES[l]
        maskF[:, l, 0] = maskF[:, l, 1] = 1.0
        maskC[:, l, 2] = maskC[:, l, 3] = 1.0
        clo[:, l, :] = [0.0, 0.0, 1.0, 1.0]
        chi[:, l, :] = [fw - 1, fh - 1, fw, fh]
        shi[:, l, :] = [fw - w, fh - w, 1e9, 1e9]
    inv4s = np.zeros((128, NLEV), np.float32)
    ninv4s = np.zeros((128, NLEV), np.float32)
    rowofs_c = np.zeros((128, NLEV), np.float32)
    for l in range(NLEV):
        inv4s[:, l] = 1.0 / (4.0 * STRIDES[l])
        ninv4s[:, l] = -1.0 / (4.0 * STRIDES[l])
        rowofs_c[:, l] = float(ROWOFS[l])
    hgat = np.zeros((128, len(GATHERS)), np.float32)
    for gi, (l, ht, hb, blk, off) in enumerate(GATHERS):
        hgat[0:64, gi] = ht
        hgat[64:128, gi] = hb
    iota_cell = np.tile(IOTA_NP, (128, 1)).astype(np.float32)
    h_cell = np.zeros((128, NCELL), np.float32)
    h_cell[0:64, :] = H_TOP
    h_cell[64:128, :] = H_BOT
    cconst = np.tile(np.arange(C, dtype=np.float32), (128, 1))
    constl = np.tile(np.arange(NLEV, dtype=np.float32), (128, 1))
    constm1 = np.full((128, 1), -1.0, np.float32)
    epsc = np.full((128, 1), EPS, np.float32)

    consts = np.concatenate(
        [recip.reshape(128, -1), maskF.reshape(128, -1), maskC.reshape(128, -1),
         clo.reshape(128, -1), chi.reshape(128, -1), shi.reshape(128, -1),
         inv4s, ninv4s, rowofs_c, hgat, iota_cell, h_cell,
         cconst, constl, constm1, epsc], axis=1)
    t_consts = nc.inline_tensor(consts, "c_all")
    NCONST = consts.shape[1]

    with tile.TileContext(nc) as tc, ExitStack() as ctx:
        pc = ctx.enter_context(tc.tile_pool(name="pc", bufs=1))

        CST = pc.tile([128, NCONST], f32)
        nc.sync.dma_start(CST[:], t_consts[:])
        off = 0
        def _cv(n):
            nonlocal off
            v = CST[:, off:off + n]
            off += n
            return v
        RECIP = _cv(20); MASKF = _cv(20); MASKC = _cv(20)
        CLO = _cv(20); CHI = _cv(20); SHI = _cv(20)
        INV4S = _cv(5); NINV4S = _cv(5); ROWOF = _cv(5)
        HGAT = _cv(len(GATHERS))
        IOTA = _cv(NCELL); HCELL = _cv(NCELL)
        CCONST = _cv(C); CONSTL = _cv(5); CONSTM1 = _cv(1); EPSC = _cv(1)

        GT = pc.tile([128, 5], f32)
        nc.scalar.dma_start(GT[0:64, :], gt_b[:])
        nc.scalar.dma_start(GT[64:128, :], gt_b[:])

        # ---- critical path: xs/ys -> row indices -> gathers ---------------
        # shrunk box: lo = 0.6*b + 0.4*b_opp (only x1,y1 cols needed early,
        # but compute all 4 in two wide ops)
        Q = pc.tile([128, 4], f32)
        TMPQ = pc.tile([128, 4], f32)
        nc.vector.tensor_scalar(TMPQ[:, 0:1], GT[:, 2:3], 0.4, None, OP.mult)
        nc.vector.tensor_scalar(TMPQ[:, 1:2], GT[:, 3:4], 0.4, None, OP.mult)
        nc.vector.tensor_scalar(TMPQ[:, 2:3], GT[:, 0:1], 0.4, None, OP.mult)
        nc.vector.tensor_scalar(TMPQ[:, 3:4], GT[:, 1:2], 0.4, None, OP.mult)
        nc.vector.scalar_tensor_tensor(Q[:, 0:1], GT[:, 0:1], 0.6, TMPQ[:, 0:1], OP.mult, OP.add)
        nc.vector.scalar_tensor_tensor(Q[:, 1:2], GT[:, 1:2], 0.6, TMPQ[:, 1:2], OP.mult, OP.add)
        nc.vector.scalar_tensor_tensor(Q[:, 2:3], GT[:, 2:3], 0.6, TMPQ[:, 2:3], OP.mult, OP.add)
        nc.vector.scalar_tensor_tensor(Q[:, 3:4], GT[:, 3:4], 0.6, TMPQ[:, 3:4], OP.mult, OP.add)

        V = pc.tile([128, 20], f32)
        nc.vector.tensor_tensor(
            out=V[:].rearrange("g (l j) -> g l j", j=4),
            in0=Q[:, None, :].to_broadcast([128, NLEV, 4]),
            in1=RECIP.rearrange("g (l j) -> g l j", j=4),
            op=OP.mult)
        # xs/ys: robust floor (cast may round-to-nearest), clamp [0, f-w]
        VIXY = pc.tile([128, 10], i32)
        V3 = V[:].rearrange("g (l j) -> g l j", j=4)
        VXY3 = V3[:, :, 0:2]
        nc.vector.tensor_copy(VIXY[:].rearrange("g (l j) -> g l j", j=2), VXY3)
        SVXY = pc.tile([128, 10], f32)
        nc.vector.tensor_copy(SVXY[:], VIXY[:])
        GGXY = pc.tile([128, 10], f32)
        nc.vector.tensor_tensor(out=GGXY[:].rearrange("g (l j) -> g l j", j=2),
                                in0=SVXY[:].rearrange("g (l j) -> g l j", j=2),
                                in1=VXY3, op=OP.is_gt)
        nc.vector.tensor_tensor(out=SVXY[:], in0=SVXY[:], in1=GGXY[:], op=OP.subtract)
        nc.vector.tensor_scalar(SVXY[:], SVXY[:], 0.0, None, OP.max)
        SHI2 = SHI.rearrange("g (l j) -> g l j", j=4)[:, :, 0:2]
        nc.vector.tensor_tensor(out=SVXY[:].rearrange("g (l j) -> g l j", j=2),
                                in0=SVXY[:].rearrange("g (l j) -> g l j", j=2),
                                in1=SHI2, op=OP.min)
        SV2 = SVXY[:].rearrange("g (l j) -> g l j", j=2)
        xsv = SV2[:, :, 0]
        ysv = SV2[:, :, 1]
        XSO = pc.tile([128, 5], f32)
        nc.vector.tensor_tensor(out=XSO[:], in0=xsv, in1=ROWOF, op=OP.add)
        NG = len(GATHERS)
        RIF = pc.tile([128, NG], f32)
        RII = pc.tile([128, NG], i32)
        CR = pc.tile([128, FREE], f32)
        # per level: row indices, cast, then that level's gathers immediately
        lev_gis = [[gi for gi, g in enumerate(GATHERS) if g[0] == l]
                   for l in range(NLEV)]
        for l in range(NLEV):
            gis = lev_gis[l]
            g0, g1 = gis[0], gis[-1] + 1
            ng = g1 - g0
            nc.vector.tensor_tensor(
                out=RIF[:, g0:g1], in0=HGAT[:, g0:g1],
                in1=ysv[:, l:l + 1].to_broadcast([128, ng]), op=OP.add)
            nc.vector.scalar_tensor_tensor(
                RIF[:, g0:g1], RIF[:, g0:g1], float(FS[l][1]),
                XSO[:, l:l + 1].to_broadcast([128, ng]), OP.mult, OP.add)
            nc.vector.tensor_copy(RII[:, g0:g1], RIF[:, g0:g1])
            for gi in gis:
                _, ht, hb, blk, croff = GATHERS[gi]
                nc.gpsimd.indirect_dma_start(
                    out=CR[:, croff:croff + blk], out_offset=None, in_=cr_b[:],
                    in_offset=bass.IndirectOffsetOnAxis(ap=RII[:, gi:gi + 1], axis=0))

        # ---- rest of box math (overlaps gathers) --------------------------
        VI = pc.tile([128, 20], i32)
        nc.vector.tensor_copy(VI[:], V[:])
        VF = pc.tile([128, 20], f32)
        nc.vector.tensor_copy(VF[:], VI[:])
        GG = pc.tile([128, 20], f32)
        nc.vector.tensor_tensor(out=GG[:], in0=VF[:], in1=V[:], op=OP.is_gt)
        LL = pc.tile([128, 20], f32)
        nc.vector.tensor_tensor(out=LL[:], in0=VF[:], in1=V[:], op=OP.is_lt)
        nc.vector.tensor_tensor(out=GG[:], in0=GG[:], in1=MASKF, op=OP.mult)
        nc.vector.tensor_tensor(out=LL[:], in0=LL[:], in1=MASKC, op=OP.mult)
        VR = pc.tile([128, 20], f32)
        nc.vector.tensor_tensor(out=VR[:], in0=VF[:], in1=GG[:], op=OP.subtract)
        nc.vector.tensor_tensor(out=VR[:], in0=VR[:], in1=LL[:], op=OP.add)
        nc.vector.tensor_tensor(out=VR[:], in0=VR[:], in1=CLO, op=OP.max)
        nc.vector.tensor_tensor(out=VR[:], in0=VR[:], in1=CHI, op=OP.min)
        VR3 = VR[:].rearrange("g (l j) -> g l j", j=4)
        x1v, y1v, x2v, y2v = VR3[:, :, 0], VR3[:, :, 1], VR3[:, :, 2], VR3[:, :, 3]

        EX = pc.tile([128, 5], f32)
        nc.vector.tensor_tensor(out=EX[:], in0=x1v, in1=x2v, op=OP.is_equal)
        EY = pc.tile([128, 5], f32)
        nc.vector.tensor_tensor(out=EY[:], in0=y1v, in1=y2v, op=OP.is_equal)
        EMX = pc.tile([128, 5], f32)
        nc.vector.tensor_tensor(out=EMX[:], in0=EX[:], in1=EY[:], op=OP.max)
        DX = pc.tile([128, 5], f32)
        nc.vector.tensor_tensor(out=DX[:], in0=x2v, in1=x1v, op=OP.subtract)
        DY = pc.tile([128, 5], f32)
        nc.vector.tensor_tensor(out=DY[:], in0=y2v, in1=y1v, op=OP.subtract)
        DN = pc.tile([128, 5], f32)
        nc.vector.tensor_tensor(out=DN[:], in0=DX[:], in1=DY[:], op=OP.mult)
        nc.vector.tensor_scalar(DN[:], DN[:], 1.0, None, OP.max)
        RECDN = pc.tile([128, 5], f32)
        nc.vector.reciprocal(RECDN[:], DN[:])

        LBL = pc.tile([128, 1], f32)
        nc.vector.tensor_scalar(LBL[:], GT[:, 4:5], 0.0, float(C - 1), OP.max, OP.min)
        ONEHOT = pc.tile([128, C], f32)
        nc.vector.tensor_tensor(out=ONEHOT[:], in0=CCONST,
                                in1=LBL[:, 0:1].to_broadcast([128, C]), op=OP.is_equal)
        SABS = pc.tile([128, 1], f32)
        nc.vector.tensor_reduce(SABS[:], GT[:, 0:4], axis=AX.X, op=OP.add,
                                apply_absolute_value=True)
        NV = pc.tile([128, 1], i32)
        nc.vector.tensor_scalar(NV[:], SABS[:], 0.0, None, OP.is_le)

        # masks [128, NCELL]
        XSI = pc.tile([128, NCELL], f32)
        HYC = pc.tile([128, NCELL], f32)
        MXY = pc.tile([128, NCELL], f32)
        TMC = pc.tile([128, NCELL], f32)
        TMC2 = pc.tile([128, NCELL], f32)
        for l in range(NLEV):
            c0, c1 = CELL_R[l]
            ncl = c1 - c0
            nc.vector.tensor_tensor(
                out=XSI[:, c0:c1], in0=IOTA[:, c0:c1],
                in1=xsv[:, l:l + 1].to_broadcast([128, ncl]), op=OP.add)
            nc.vector.tensor_tensor(
                out=HYC[:, c0:c1], in0=HCELL[:, c0:c1],
                in1=ysv[:, l:l + 1].to_broadcast([128, ncl]), op=OP.add)
            nc.vector.tensor_scalar(MXY[:, c0:c1], XSI[:, c0:c1],
                                    VR[:, 4 * l:4 * l + 1], None, OP.is_ge)
            nc.vector.tensor_scalar(TMC[:, c0:c1], XSI[:, c0:c1],
                                    VR[:, 4 * l + 2:4 * l + 3], None, OP.is_lt)
            nc.vector.tensor_scalar(TMC2[:, c0:c1], HYC[:, c0:c1],
                                    VR[:, 4 * l + 1:4 * l + 2], None, OP.is_ge)
        nc.vector.tensor_tensor(out=MXY[:], in0=MXY[:], in1=TMC[:], op=OP.mult)
        for l in range(NLEV):
            c0, c1 = CELL_R[l]
            nc.vector.tensor_scalar(TMC[:, c0:c1], HYC[:, c0:c1],
                                    VR[:, 4 * l + 3:4 * l + 4], None, OP.is_lt)
        nc.vector.tensor_tensor(out=TMC2[:], in0=TMC2[:], in1=TMC[:], op=OP.mult)
        nc.vector.tensor_tensor(out=MXY[:], in0=MXY[:], in1=TMC2[:], op=OP.mult)

        # iou target prep (cheap, feeds ACT Relus early)
        SXS = pc.tile([128, NCELL], f32)
        nc.vector.tensor_scalar(SXS[:], XSI[:], 0.5, 0.25, OP.add, OP.mult)
        SYS = pc.tile([128, NCELL], f32)
        nc.vector.tensor_scalar(SYS[:], HYC[:], 0.5, 0.25, OP.add, OP.mult)
        B0N = pc.tile([128, 5], f32)
        nc.vector.tensor_tensor(out=B0N[:], in0=GT[:, 0:1].to_broadcast([128, 5]),
                                in1=NINV4S, op=OP.mult)
        B1N = pc.tile([128, 5], f32)
        nc.vector.tensor_tensor(out=B1N[:], in0=GT[:, 1:2].to_broadcast([128, 5]),
                                in1=NINV4S, op=OP.mult)
        B2P = pc.tile([128, 5], f32)
        nc.vector.tensor_tensor(out=B2P[:], in0=GT[:, 2:3].to_broadcast([128, 5]),
                                in1=INV4S, op=OP.mult)
        B3P = pc.tile([128, 5], f32)
        nc.vector.tensor_tensor(out=B3P[:], in0=GT[:, 3:4].to_broadcast([128, 5]),
                                in1=INV4S, op=OP.mult)

        # ---- ACT normalization: strided CR -> packed LN1 / SQ / RP4 -------
        CR3c = CR[:].rearrange("g (r c) -> g r c", c=RC)
        LN1 = pc.tile([128, NCELL * C], f32)
        LN13 = LN1[:].rearrange("g (k c) -> g k c", c=C)
        SQ = pc.tile([128, NCELL * C], f32)
        SQ3 = SQ[:].rearrange("g (k c) -> g k c", c=C)
        RP4 = pc.tile([128, NCELL * 4], f32)
        RP43 = RP4[:].rearrange("g (k j) -> g k j", j=4)
        act_runs = [(9 * k, 9, 756 * k) for k in range(5)] + [
            (45, 5, 3780), (50, 5, 3780 + 64 * RC), (55, 5, 9576),
            (60, 3, 9996), (63, 3, 9996 + 32 * RC),
            (66, 2, 12936), (68, 2, 13104)]
        for (cell0, ncl, croff) in act_runs:
            r0 = croff // RC
            cls_v = CR3c[:, r0:r0 + ncl, 0:C]
            nc.scalar.activation(LN13[:, cell0:cell0 + ncl, :], cls_v,
                                 AF.Ln, bias=1.0, scale=-1.0)
            nc.scalar.activation(SQ3[:, cell0:cell0 + ncl, :], cls_v, AF.Square)
            nc.scalar.activation(RP43[:, cell0:cell0 + ncl, :],
                                 CR3c[:, r0:r0 + ncl, C:RC], AF.Copy, scale=4.0)

        # iou targets on ACT (per-partition bias trick)
        TL = pc.tile([128, NCELL], f32)
        TR = pc.tile([128, NCELL], f32)
        TT = pc.tile([128, NCELL], f32)
        TB = pc.tile([128, NCELL], f32)
        for l in range(NLEV):
            c0, c1 = CELL_R[l]
            nc.scalar.activation(TL[:, c0:c1], SXS[:, c0:c1], AF.Relu,
                                 bias=B0N[:, l:l + 1], scale=1.0)
            nc.scalar.activation(TR[:, c0:c1], SXS[:, c0:c1], AF.Relu,
                                 bias=B2P[:, l:l + 1], scale=-1.0)
            nc.scalar.activation(TT[:, c0:c1], SYS[:, c0:c1], AF.Relu,
                                 bias=B1N[:, l:l + 1], scale=1.0)
            nc.scalar.activation(TB[:, c0:c1], SYS[:, c0:c1], AF.Relu,
                                 bias=B3P[:, l:l + 1], scale=-1.0)

        # ---- iou elementwise chain on Pool (frees DVE) --------------------
        rp0 = RP43[:, :, 0]
        rp1 = RP43[:, :, 1]
        rp2 = RP43[:, :, 2]
        rp3 = RP43[:, :, 3]
        TLR = pc.tile([128, NCELL], f32)
        nc.gpsimd.tensor_tensor(out=TLR[:], in0=TL[:], in1=TR[:], op=OP.add)
        TTB = pc.tile([128, NCELL], f32)
        nc.gpsimd.tensor_tensor(out=TTB[:], in0=TT[:], in1=TB[:], op=OP.add)
        TAREA = pc.tile([128, NCELL], f32)
        nc.gpsimd.tensor_tensor(out=TAREA[:], in0=TLR[:], in1=TTB[:], op=OP.mult)
        W1 = pc.tile([128, NCELL], f32)
        nc.vector.tensor_tensor(out=W1[:], in0=rp0, in1=TL[:], op=OP.min)
        W2 = pc.tile([128, NCELL], f32)
        nc.vector.tensor_tensor(out=W2[:], in0=rp2, in1=TR[:], op=OP.min)
        WI = pc.tile([128, NCELL], f32)
        nc.gpsimd.tensor_tensor(out=WI[:], in0=W1[:], in1=W2[:], op=OP.add)
        H1 = pc.tile([128, NCELL], f32)
        nc.vector.tensor_tensor(out=H1[:], in0=rp1, in1=TT[:], op=OP.min)
        H2 = pc.tile([128, NCELL], f32)
        nc.vector.tensor_tensor(out=H2[:], in0=rp3, in1=TB[:], op=OP.min)
        HI = pc.tile([128, NCELL], f32)
        nc.gpsimd.tensor_tensor(out=HI[:], in0=H1[:], in1=H2[:], op=OP.add)
        PLR = pc.tile([128, NCELL], f32)
        nc.gpsimd.tensor_tensor(out=PLR[:], in0=rp0, in1=rp2, op=OP.add)
        PTB = pc.tile([128, NCELL], f32)
        nc.gpsimd.tensor_tensor(out=PTB[:], in0=rp1, in1=rp3, op=OP.add)
        PAREA = pc.tile([128, NCELL], f32)
        nc.gpsimd.tensor_tensor(out=PAREA[:], in0=PLR[:], in1=PTB[:], op=OP.mult)
        AI = pc.tile([128, NCELL], f32)
        nc.gpsimd.tensor_tensor(out=AI[:], in0=WI[:], in1=HI[:], op=OP.mult)
        AU = pc.tile([128, NCELL], f32)
        nc.gpsimd.tensor_tensor(out=AU[:], in0=TAREA[:], in1=PAREA[:], op=OP.add)
        nc.gpsimd.tensor_tensor(out=AU[:], in0=AU[:], in1=AI[:], op=OP.subtract)
        nc.gpsimd.tensor_tensor(out=AI[:], in0=AI[:],
                                in1=EPSC[:, 0:1].to_broadcast([128, NCELL]), op=OP.add)
        nc.gpsimd.tensor_tensor(out=AU[:], in0=AU[:],
                                in1=EPSC[:, 0:1].to_broadcast([128, NCELL]), op=OP.add)
        LNAI = pc.tile([128, NCELL], f32)
        nc.scalar.activation(LNAI[:], AI[:], AF.Ln)
        LNAU = pc.tile([128, NCELL], f32)
        nc.scalar.activation(LNAU[:], AU[:], AF.Ln)
        LNR = pc.tile([128, NCELL], f32)
        nc.gpsimd.tensor_tensor(out=LNR[:], in0=LNAI[:], in1=LNAU[:], op=OP.subtract)

        # ---- DVE big passes: T(+S1 accum), OHM, SQP -----------------------
        T = pc.tile([128, NCELL * C], f32)
        T3 = T[:].rearrange("g (k c) -> g k c", c=C)
        OHM = pc.tile([128, NCELL * C], f32)
        OHM3 = OHM[:].rearrange("g (k c) -> g k c", c=C)
        SQP = pc.tile([128, NCELL], f32)
        NCH = 6
        S1C = pc.tile([128, NCH], f32)
        chunks = [(0, 0, 27), (0, 27, 45), (1, 45, 60), (2, 60, 66),
                  (3, 66, 68), (4, 68, 70)]
        for ci, (l, c0, c1) in enumerate(chunks):
            ncl = c1 - c0
            nc.vector.tensor_tensor(out=T[:, c0 * C:c1 * C], in0=SQ[:, c0 * C:c1 * C],
                                    in1=LN1[:, c0 * C:c1 * C], op=OP.mult)
            nc.vector.scalar_tensor_tensor(
                T3[:, c0:c1, :], T3[:, c0:c1, :], 1.0,
                MXY[:, c0:c1, None].to_broadcast([128, ncl, C]),
                OP.mult, OP.mult, accum_out=S1C[:, ci:ci + 1])
            nc.vector.tensor_tensor(
                out=OHM3[:, c0:c1, :], in0=SQ3[:, c0:c1, :],
                in1=ONEHOT[:, None, :].to_broadcast([128, ncl, C]), op=OP.mult)
            nc.vector.tensor_reduce(SQP[:, c0:c1], OHM3[:, c0:c1, :],
                                    axis=AX.X, op=OP.add)
        S1 = pc.tile([128, 5], f32)
        nc.vector.tensor_copy(S1[:], S1C[:, 1:6])
        nc.vector.tensor_tensor(out=S1[:, 0:1], in0=S1C[:, 0:1], in1=S1C[:, 1:2],
                                op=OP.add)

        # ---- psel focal terms (psel = sqrt(SQP), ln psel = 0.5 ln SQP) ----
        PSEL = pc.tile([128, NCELL], f32)
        nc.scalar.activation(PSEL[:], SQP[:], AF.Sqrt)
        LNSQP = pc.tile([128, NCELL], f32)
        nc.scalar.activation(LNSQP[:], SQP[:], AF.Ln)
        LN1P = pc.tile([128, NCELL], f32)
        nc.scalar.activation(LN1P[:], PSEL[:], AF.Ln, bias=1.0, scale=-1.0)
        SQ1P = pc.tile([128, NCELL], f32)
        nc.scalar.activation(SQ1P[:], PSEL[:], AF.Square, bias=1.0, scale=-1.0)
        M1D = pc.tile([128, NCELL], f32)   # = 2 * m1
        nc.vector.tensor_tensor(out=M1D[:], in0=SQ1P[:], in1=LNSQP[:], op=OP.mult)
        M2 = pc.tile([128, NCELL], f32)
        nc.vector.tensor_tensor(out=M2[:], in0=SQP[:], in1=LN1P[:], op=OP.mult)

        # SMALL = 0.125*(2 m1) - 0.75*m2 + lnr ; S2_l = sum mask*SMALL
        SM = pc.tile([128, NCELL], f32)
        nc.vector.scalar_tensor_tensor(SM[:], M1D[:], 0.125, LNR[:], OP.mult, OP.add)
        nc.vector.scalar_tensor_tensor(SM[:], M2[:], -0.75, SM[:], OP.mult, OP.add)
        S2 = pc.tile([128, 5], f32)
        for l in range(NLEV):
            c0, c1 = CELL_R[l]
            nc.vector.scalar_tensor_tensor(
                SM[:, c0:c1], SM[:, c0:c1], 1.0, MXY[:, c0:c1],
                OP.mult, OP.mult, accum_out=S2[:, l:l + 1])

        LS = pc.tile([128, 5], f32)
        nc.vector.scalar_tensor_tensor(LS[:], S1[:], 0.75, S2[:], OP.mult, OP.add)
        nc.vector.tensor_tensor(out=LS[:], in0=LS[:], in1=RECDN[:], op=OP.mult)

        TMH = pc.tile([64, 5], f32)
        nc.sync.dma_start(TMH[:], LS[64:128, :])
        LC = pc.tile([64, 5], f32)
        nc.vector.tensor_tensor(out=LC[:], in0=LS[0:64, :], in1=TMH[:], op=OP.add)
        LOSS = pc.tile([64, 5], f32)
        nc.vector.scalar_tensor_tensor(LOSS[:], EMX[0:64, :], BIG, LC[:],
                                       OP.mult, OP.subtract)

        MBEST = pc.tile([64, 1], f32)
        nc.vector.tensor_copy(MBEST[:], LOSS[:, 0:1])
        IDX = pc.tile([64, 1], f32)
        nc.vector.memset(IDX[:], 0.0)
        for l in range(1, NLEV):
            CMP = pc.tile([64, 1], i32, tag=f"cmp{l}")
            nc.vector.tensor_tensor(out=CMP[:], in0=LOSS[:, l:l + 1], in1=MBEST[:], op=OP.is_lt)
            nc.vector.copy_predicated(IDX[:], CMP[:], CONSTL[0:64, l:l + 1])
            nc.vector.tensor_tensor(out=MBEST[:], in0=MBEST[:], in1=LOSS[:, l:l + 1], op=OP.min)
        nc.vector.copy_predicated(IDX[:], NV[0:64, :], CONSTM1[0:64, :])
        IDXI = pc.tile([64, 1], i32)
        nc.vector.tensor_copy(IDXI[:], IDX[:])
        nc.sync.dma_start(out_lvl.ap()[:, None], IDXI[:])

    nc.compile()
    return nc


_NC_CACHE = None


def _get_nc():
    global _NC_CACHE
    if _NC_CACHE is None:
        _NC_CACHE = build_nc(num_devices=8)
    return _NC_CACHE


def make_in_maps(cls_pred, regr_pred, gt_boxes):
    B = cls_pred.shape[0]
    comb = np.zeros((B, NPAD, RC), np.float32)
    comb[:, :NLOC, 0:C] = cls_pred
    comb[:, :NLOC, C:RC] = regr_pred * RSCALE
    return [
        {"cr_b": np.ascontiguousarray(comb[b]),
         "gt_b": np.ascontiguousarray(gt_boxes[b], dtype=np.float32)}
        for b in range(B)
    ]


def kernel(cls_pred, regr_pred, feature_shapes, gt_boxes):
    from concourse.bass_utils import run_bass_kernel_spmd

    B = cls_pred.shape[0]
    assert B == 8 and cls_pred.shape[1] == NLOC and cls_pred.shape[2] == C
    nc = _get_nc()
    in_maps = make_in_maps(cls_pred, regr_pred, gt_boxes)
    res = run_bass_kernel_spmd(nc, in_maps, list(range(B)))
    out = np.stack([np.asarray(res.results[b]["out_lvl"]).reshape(G) for b in range(B)])
    return out.reshape(-1).astype(np.int32)
